# revision 23
# baseline (speedup 1.0000x reference)
"""MoE routing kernel (2 experts, D=128 -> H=512 -> O=2) for 8 Trainium2 cores.

Strategy: expert-sorted sharding. The routing decision (a 128-dim dot vs a
threshold) is computed host-side as part of choosing the data distribution;
samples are stable-partitioned by expert, padded so every core receives the
identical layout (kb0 expert-0 blocks followed by kb1 expert-1 blocks of 512
samples), and uploaded pre-transposed in bf16. Each core then runs a pure
dense single-expert MLP per block:

  per 512-sample block (expert e fixed at compile time):
    DMA xT tile [128d, 512b] bf16 (batched 4 blocks/transfer)
    PE  layer-1: 4 matmuls (w1 j-tiles stationary, xT moving) -> z PSUM
    ACT/DVE: relu(z + b1) -> h SBUF bf16   (two fused [128,1024] ops)
    PE  layer-2: 4 CONCURRENT column-tiled matmuls: each j-chunk's w2 slice
        [128k, 2o] is stationary at tile_position (0, 32j), so the four
        512-column streams overlap in disjoint 32-column groups of the PE
        array.  The four partial outputs land at PSUM partitions {32j, 32j+1}
        and are summed on the HOST (with b2) after the gather -- this cuts
        layer-2 PE time ~4x vs a padded M=128 matmul per j-chunk.
    ACT/DVE: one [98, 2, 512] PSUM->SBUF copy per block pair, then a DMA per
        4-block group per column-group.

Emission is software-pipelined (layer-1 of block n before layer-2 of block
n-1) so the PE never waits on the relu engines, and warmup matmuls ramp the
PE to its top p-state while the first DMAs are in flight.  The host gathers
per-core [8, n] partial outputs, reduces the 4 column-group partials, adds
b2, and scatters rows back through the inverse permutation.
"""

import numpy as np
import ml_dtypes

import concourse.bacc as bacc
import concourse.mybir as mybir
import concourse.tile as tile
from concourse.bass_utils import run_bass_kernel_spmd

F32 = mybir.dt.float32
BF16 = mybir.dt.bfloat16
BF16_NP = ml_dtypes.bfloat16

N_CORES = 8
D = 128
H = 512
E = 2
O = 2
NJ = H // 128         # 4 hidden k-tiles of 128 per expert
BLK = 512             # samples per block
WCOL = H + NJ * O     # per-expert packed weights (w1t | w2c [j,o] columns)


def _build_program(nb: int, kb0: int):
    """Per-core program: nb blocks of 512; first kb0 blocks use expert 0."""
    nc = bacc.Bacc(
        "TRN2",
        target_bir_lowering=False,
        debug=False,
        enable_asserts=False,
        num_devices=1,
    )

    n_shard = nb * BLK
    xt = nc.dram_tensor("xt", [D, n_shard], BF16, kind="ExternalInput").ap()
    # whead = weights of the first-used expert (whead0 = its first j-tile,
    # tiny, so block 0's first matmul starts as early as possible), wtail =
    # the other expert's
    whead0 = nc.dram_tensor("whead0", [D, 128], BF16, kind="ExternalInput").ap()
    whead1 = nc.dram_tensor("whead1", [D, H - 128], BF16, kind="ExternalInput").ap()
    whead2 = nc.dram_tensor("whead2", [D, WCOL - H], BF16, kind="ExternalInput").ap()
    wtail = nc.dram_tensor("wtail", [D, WCOL], BF16, kind="ExternalInput").ap()
    cf32 = nc.dram_tensor("cf32", [D, E * NJ], F32, kind="ExternalInput").ap()
    # NJ*O rows of layer-2 partials; host sums the NJ groups
    out = nc.dram_tensor("out", [NJ * O, n_shard], F32, kind="ExternalOutput").ap()

    with tile.TileContext(nc) as tc:
        _body(tc, nb, kb0, xt, whead0, whead1, whead2, wtail, cf32, out)

    nc.compile()
    return nc


def _body(tc, nb, kb0, xt, whead0, whead1, whead2, wtail, cf32, out):
    nc = tc.nc
    Relu = mybir.ActivationFunctionType.Relu
    Copy = mybir.ActivationFunctionType.Copy
    Alu = mybir.AluOpType
    e_first = 0 if kb0 > 0 else 1
    OG = 8  # blocks per out-DMA group

    # x chunk schedule: small first chunks so block 0 lands fast, then
    # steady 4-block chunks.  ALL x DMAs are emitted up-front so they sit
    # ahead of every out-DMA in the sync HWDGE FIFO (an out-DMA waiting on
    # its copy semaphore would otherwise starve the x stream mid-kernel).
    chunks = [1, 1, 2]
    while sum(chunks) < nb:
        chunks.append(min(4, nb - sum(chunks)))
    chunk_base = [sum(chunks[:i]) for i in range(len(chunks))]

    with (
        tc.tile_pool(name="consts", bufs=1) as cpool,
        tc.tile_pool(name="xs", bufs=len(chunks)) as x_pool,
        # separate tiles per relu half: a shared h tile would make the tile
        # tracker serialize the ACT and DVE relu writes (tile-granular WAW)
        tc.tile_pool(name="h", bufs=6) as h_pool,
        tc.tile_pool(name="os", bufs=2) as o_pool,
        tc.tile_pool(name="zp", bufs=3, space="PSUM") as zp_pool,
        tc.tile_pool(name="op", bufs=2, space="PSUM") as op_pool,
    ):
        # No PE warmup: any SBUF source for dummy matmuls is gated behind
        # the ~5us engine library preambles plus sem hops, so real matmuls
        # start as soon as x0+whead0 land and ramp the HAM clock during
        # real work (~1.7us one-time cold penalty).

        # Block-0 gates: x chunk 0 first (it has the longest chain after
        # it), then the first j-tile of the first expert and b1; the rest
        # of the x stream follows.  Remaining weights ride the ACT queue
        # (idle at startup).
        xqs = []

        def emit_x(ci):
            xq = x_pool.tile([D, chunks[ci], BLK], BF16, name="xq")
            xqs.append(xq)
            nc.sync.dma_start(
                xq.rearrange("p t b -> p (t b)"),
                xt[:, chunk_base[ci] * BLK : (chunk_base[ci] + chunks[ci]) * BLK],
            )

        emit_x(0)
        wh_sb = cpool.tile([D, WCOL], BF16)
        nc.sync.dma_start(wh_sb[:, 0:128], whead0)
        cf_sb = cpool.tile([D, E * NJ], F32)
        nc.sync.dma_start(cf_sb[:], cf32)
        for ci in range(1, len(chunks)):
            emit_x(ci)

        nc.scalar.dma_start(wh_sb[:, 128:H], whead1)
        nc.scalar.dma_start(wh_sb[:, H:WCOL], whead2)
        wt_sb = cpool.tile([D, WCOL], BF16)
        nc.scalar.dma_start(wt_sb[:], wtail)
        wsb = [wh_sb, wt_sb] if e_first == 0 else [wt_sb, wh_sb]
        w1t_of = lambda e: wsb[e][:, 0:H]
        # layer-2 stationary for chunk j: [128 k, 2 o] slice
        w2c_of = lambda e, j: wsb[e][:, H + O * j : H + O * (j + 1)]
        b1c_sb = cf_sb

        hs = [None] * nb
        ops = [None] * nb  # per-block PSUM layer-2 partial tiles
        osbt = [None]  # current out-DMA group SBUF tile
        ci_of = [None] * nb
        for ci, cn in enumerate(chunks):
            for k in range(cn):
                ci_of[chunk_base[ci] + k] = ci
        # out-DMA groups: 8 blocks in steady state, shrinking toward the
        # end so the final DMA chains are short
        gsz = []
        while nb - sum(gsz) > 8:
            gsz.append(8)
        rem = nb - sum(gsz)
        while rem > 4:
            gsz.append(4)
            rem -= 4
        if rem > 1:
            gsz.append(rem - 1)
            rem = 1
        gsz.append(rem)
        g_of = {}
        b0 = 0
        for gi, sz in enumerate(gsz):
            for k in range(sz):
                g_of[b0 + k] = (gi, k, sz, b0)
            b0 += sz

        def emit_l1(bi, half):
            e = 0 if bi < kb0 else 1
            ci = ci_of[bi]
            xq = xqs[ci]
            xq_base = chunk_base[ci]
            if half == 0:
                hs[bi] = [None, None]
            zp = zp_pool.tile([D, 2, BLK], F32, name="zp")
            for k in range(2):
                j = half * 2 + k
                nc.tensor.matmul(
                    zp[:, k, :],
                    lhsT=w1t_of(e)[:, j * 128 : (j + 1) * 128],
                    rhs=xq[:, bi - xq_base, :],
                    start=True,
                    stop=True,
                )
            # relu(z + b1) -> h bf16; ACT for half 0, DVE for half 1
            h = h_pool.tile([D, 2, BLK], BF16, name="h")
            hs[bi][half] = h
            j0 = half * 2
            if half == 0:
                nc.scalar.activation(
                    h[:],
                    zp[:],
                    Relu,
                    bias=b1c_sb[:, e * NJ + j0 : e * NJ + j0 + 1],
                    scale=1.0,
                )
            else:
                nc.vector.tensor_scalar(
                    out=h[:],
                    in0=zp[:],
                    scalar1=b1c_sb[:, e * NJ + j0 : e * NJ + j0 + 1],
                    scalar2=0.0,
                    op0=Alu.add,
                    op1=Alu.max,
                )

        def emit_l2(bi):
            # 4 column-tiled matmuls, concurrent in disjoint 32-col groups
            e = 0 if bi < kb0 else 1
            hh = hs[bi]
            hs[bi] = None
            op = op_pool.tile([D, BLK], F32, name="op")
            ops[bi] = op
            for j in range(NJ):
                nc.tensor.matmul(
                    op[32 * j : 32 * j + O, :],
                    lhsT=w2c_of(e, j),
                    rhs=hh[j // 2][:, j % 2, :],
                    start=True,
                    stop=True,
                    tile_position=(0, 32 * j),
                )

        def emit_out(b):
            # one PSUM->SBUF copy per block (partials at partitions
            # {32j, 32j+1}); DMA per group per column-group.
            # Copies go 2:1 to ACT:DVE to balance against the relu halves.
            g, t, sz, gb = g_of[b]
            if t == 0:
                osbt[0] = o_pool.tile([D, sz, BLK], F32, name="osb")
            osb = osbt[0]
            np98 = 32 * (NJ - 1) + O
            src = ops[b][0:np98, :]
            ops[b] = None
            dst = osb[0:np98, t, :]
            if b % 3 == 2:
                nc.vector.tensor_scalar(
                    out=dst, in0=src, scalar1=0.0, scalar2=None, op0=Alu.add
                )
            else:
                nc.scalar.activation(dst, src, Copy, bias=0.0, scale=1.0)
            # flush at group end; the final flush splits its 4 DMAs across
            # the Sync and ACT queues to shorten the tail (the ACT queue is
            # busy mid-kernel but free at the end)
            if t != sz - 1:
                return
            for j in range(NJ):
                dq = nc.scalar if (b == nb - 1 and j >= 2) else nc.sync
                dq.dma_start(
                    out[O * j : O * (j + 1), gb * BLK : (gb + sz) * BLK],
                    osb[32 * j : 32 * j + O, :, :].rearrange(
                        "p t b -> p (t b)"
                    ),
                )

        # software-pipelined emission with a 2-block lag: L2(n-2) runs
        # between the halves of L1(n), so layer-2 never waits on the relu
        # latency chain and the zp-tile reuse loop stays loose
        for bi in range(nb):
            emit_l1(bi, 0)
            if bi >= 2:
                emit_l2(bi - 2)
            emit_l1(bi, 1)
            if bi >= 2:
                emit_out(bi - 2)
        for bi in (nb - 2, nb - 1):
            emit_l2(bi)
            emit_out(bi)


_PROG_CACHE = {}


def _get_program(nb, kb0):
    key = (nb, kb0)
    if key not in _PROG_CACHE:
        _PROG_CACHE[key] = _build_program(nb, kb0)
    return _PROG_CACHE[key]


def kernel(x, w1, b1, w2, b2, prototypes, _trace=False):
    x = np.ascontiguousarray(np.asarray(x, np.float32))
    w1 = np.asarray(w1, np.float32)
    b1 = np.asarray(b1, np.float32)
    w2 = np.asarray(w2, np.float32)
    b2 = np.asarray(b2, np.float32)
    p = np.asarray(prototypes, np.float64)
    btot = x.shape[0]

    # host routing (argmin over squared distance == threshold test on the
    # projection onto p1-p0); expert 0 wins ties like argmin does
    rvec = p[1] - p[0]
    thr = (p[1] @ p[1] - p[0] @ p[0]) / 2.0
    q = x.astype(np.float64) @ rvec
    t1 = q > thr
    idx0 = np.flatnonzero(~t1)
    idx1 = np.flatnonzero(t1)
    n0, n1 = idx0.size, idx1.size

    # pad each expert's block count to a multiple of 8 so all cores get the
    # same (kb0, kb1) layout and run one SPMD program
    kb0 = -(-(-(-n0 // BLK)) // N_CORES)
    kb1 = -(-(-(-n1 // BLK)) // N_CORES)
    nb = kb0 + kb1
    ns = nb * BLK  # samples per core (with padding)

    xe = np.zeros((N_CORES * ns, D), np.float32)
    e0x = x[idx0]
    e1x = x[idx1]
    c0, c1 = kb0 * BLK, kb1 * BLK
    for c in range(N_CORES):
        s0 = c * c0
        z0 = min(max(n0 - s0, 0), c0)
        if z0:
            xe[c * ns : c * ns + z0] = e0x[s0 : s0 + z0]
        s1 = c * c1
        z1 = min(max(n1 - s1, 0), c1)
        if z1:
            xe[c * ns + c0 : c * ns + c0 + z1] = e1x[s1 : s1 + z1]
    xtb = np.ascontiguousarray(xe.T.astype(BF16_NP))  # [128, 8*ns]

    # per-expert packed weights [w1t | w2c] bf16; w2c column j*O+o holds
    # w2[e, o, j*128 : (j+1)*128]
    wpk = []
    b1c = np.zeros((D, E * NJ), np.float32)
    for e in range(E):
        w2c = np.zeros((D, NJ * O), np.float32)
        for j in range(NJ):
            for o in range(O):
                w2c[:, j * O + o] = w2[e, o, j * 128 : (j + 1) * 128]
            b1c[:, e * NJ + j] = b1[e, j * 128 : (j + 1) * 128]
        wpk.append(np.concatenate([w1[e].T, w2c], axis=1).astype(BF16_NP))
    cf32 = b1c

    e_first = 0 if kb0 > 0 else 1
    nc = _get_program(nb, kb0)
    consts = dict(
        whead0=np.ascontiguousarray(wpk[e_first][:, :128]),
        whead1=np.ascontiguousarray(wpk[e_first][:, 128:H]),
        whead2=np.ascontiguousarray(wpk[e_first][:, H:]),
        wtail=wpk[1 - e_first],
        cf32=cf32,
    )
    in_maps = []
    for c in range(N_CORES):
        m = dict(consts)
        m["xt"] = np.ascontiguousarray(xtb[:, c * ns : (c + 1) * ns])
        in_maps.append(m)

    res = run_bass_kernel_spmd(
        nc, in_maps, core_ids=list(range(N_CORES)), trace=_trace
    )

    # gather: per-core [NJ*O, ns] partials -> sum the NJ column-group
    # partials, add b2, drop padding, inverse permutation
    oute = np.stack(
        [res.results[c]["out"] for c in range(N_CORES)]
    )  # [8, NJ*O, ns]
    oute = oute.reshape(N_CORES, NJ, O, ns).sum(axis=1)  # [8, O, ns]
    oute[:, :, :c0] += b2[0][None, :, None]
    oute[:, :, c0:] += b2[1][None, :, None]
    oute = oute.transpose(0, 2, 1)  # [8, ns, O]
    full = np.empty((btot, O), np.float32)
    if n0:
        full[idx0] = oute[:, :c0, :].reshape(N_CORES * c0, O)[:n0]
    if n1:
        full[idx1] = oute[:, c0:, :].reshape(N_CORES * c1, O)[:n1]
    if _trace:
        return full, res
    return full
